# revision 62
# baseline (speedup 1.0000x reference)
"""DeformableConv1d TRN2 Bass kernel (v4).

Per batch sample (one NeuronCore each, 8 cores):
  offset/mask = conv1d over x.T; pos = clip(l+off); fl/alpha; out[c,l] =
  sum_k mask*((1-a)*x[fl,c] + a*x[fl+1,c]) -- collapses to a 7-diagonal
  band: out[c,l] = sum_{s=-3..3} vv_s[l] * x[l+s, c].

v4 structure (bf16 datapath, PE-heavy band):
 - host converts x to bf16; xb (l-part, c) loaded with 4 fat DMAs and
   xT (c-part, l) loaded with 4 DMA-transposes (XBAR) -- no PE
   transposes, no converts.
 - conv computed directly as zT[l-part, j]: per l-tile, 1 bias matmul +
   6 PSUM-accumulated [128c x 6j] matmuls with shifted xT windows.
 - elementwise (fp32): pos/floor/alpha/d + sigmoid -> wf/wc; iota-compare
   builds vv2[p, si*32+t] (bf16); 13 PE shift-matmuls -> W2pre fp32
   (w2pre[p, u*32+m] = vv_{3-u}[128m+p+u-3]).
 - band: y_u[m] = xb[m] * w2pre[:,u,m] (per-partition scalar, bf16 4x
   DVE / Act scale-copy / Pool) then PSUM-accumulated shifted-identity
   matmuls: psum[c, j] += y_u[m][j+s-128t, c]; quad psums [128, 512].
 - drains psum->out_cl bf16; 8 fat output DMAs; host reshapes raw.
"""
import numpy as np

import bass_rust
import concourse.bacc as bacc
import concourse.bass as bass
import concourse.tile as tile
from concourse import mybir
from concourse.bass_utils import run_bass_kernel_spmd
import ml_dtypes

AP = bass_rust.AP
dt = mybir.dt
F32 = dt.float32
BF16 = dt.bfloat16

B, L, C, K = 8, 4096, 256, 3
P = 128
NT = L // P            # 32 l-tiles
NQ = 8                 # quads (512 l each)
ND = 7                 # diagonals s in [-3, 3]
XT_W = L + 2           # xT padded with zero col at l=-1 and l=L

# const blob layout (bf16): ident | wconv | ones | bias | shift mats
IDENT_O = 0
WCONV_O = 128
ONES_O = 164
BIAS_O = 292
ZERO_O = 298
SH_O = 426
F = 134                # band psum width: f in [0,134), l = 128m-3+f
OH_O = 426 + 13 * 128  # 7 onehot diag matrices [128, F]: OH_u[p, p+u]=1
_cache = {}


def _sh_cols():
    cols = {}
    col = SH_O
    for u in range(ND):
        cols[("m", u)] = col
        col += P
        if u != 3:
            cols[("c", u)] = col
            col += P
    col = OH_O + ND * F
    return cols, col


def _build(w_off, b_off, w_mask, b_mask, dbg=False):
    nc = bacc.Bacc("TRN2", target_bir_lowering=False, debug=False)

    x_in = nc.dram_tensor("x", [L, C], BF16, kind="ExternalInput").ap()
    xt_in = nc.dram_tensor("xt", [C, L], BF16, kind="ExternalInput").ap()
    out_d = nc.dram_tensor("out", [C, L], BF16, kind="ExternalOutput").ap()
    if dbg:
        z_d = nc.dram_tensor("z_dbg", [P, NT * 6], F32,
                             kind="ExternalOutput").ap()
        w2_d = nc.dram_tensor("w2_dbg", [P, ND * NT], F32,
                              kind="ExternalOutput").ap()
        y_d = nc.dram_tensor("y_dbg", [P, ND * C], BF16,
                             kind="ExternalOutput").ap()

    sh_cols, BW = _sh_cols()
    blob = np.zeros((P, BW), np.float32)
    blob[0:P, IDENT_O:IDENT_O + P] = np.eye(P, dtype=np.float32)
    # conv weights: wconv[c_in_g, 36] with col (g*3+dk)*6 + j
    for g in range(2):
        for dk in range(K):
            for j in range(6):
                w = w_off if j < 3 else w_mask
                blob[:, WCONV_O + (g * 3 + dk) * 6 + j] = \
                    w[j % 3, g * P:(g + 1) * P, dk]
    blob[0, ONES_O:ONES_O + P] = 1.0
    blob[0, BIAS_O:BIAS_O + 3] = np.asarray(b_off)
    blob[0, BIAS_O + 3:BIAS_O + 6] = np.asarray(b_mask)
    # shift matrices (same semantics as the v2 kernel's W2pre build)
    for u in range(ND):
        sh = u - 3
        m_ = np.zeros((P, P), np.float32)
        for p in range(P):
            if 0 <= p + sh < P:
                m_[p + sh, p] = 1.0
        blob[:, sh_cols[("m", u)]:sh_cols[("m", u)] + P] = m_
        if sh > 0:
            c_ = np.zeros((P, P), np.float32)
            for p in range(P - sh, P):
                c_[p + sh - P, p] = 1.0
            blob[:, sh_cols[("c", u)]:sh_cols[("c", u)] + P] = c_
        elif sh < 0:
            c_ = np.zeros((P, P), np.float32)
            for p in range(0, -sh):
                c_[p + sh + P, p] = 1.0
            blob[:, sh_cols[("c", u)]:sh_cols[("c", u)] + P] = c_
    for u in range(ND):
        for p in range(P):
            if p + u < F:
                blob[p, OH_O + u * F + p + u] = 1.0
    blob_bf = np.ascontiguousarray(blob.astype(ml_dtypes.bfloat16))
    blob_h = nc.inline_tensor(blob_bf, name="blob")

    A = mybir.AluOpType
    ACT = mybir.ActivationFunctionType

    # static greedy schedule for the 224 scaling ops; u=3 pinned to DVE
    # (its band matmuls come first), per-quad balanced greedy for the rest
    # so every engine finishes a quad's share at the same time.
    est = {"D": 127.0, "A": 440.0, "Pl": 522.0}
    y_eng = {}
    for q in range(NQ):
        load = {"D": 1100.0, "A": 0.0, "Pl": 600.0}
        for m in range(4 * q, 4 * q + 4):
            y_eng[(3, m)] = "D"
            load["D"] += est["D"]
        for u in (2, 4, 1, 5, 0, 6):
            for m in range(4 * q, 4 * q + 4):
                e = min(est, key=lambda k: load[k] + est[k])
                y_eng[(u, m)] = e
                load[e] += est[e]

    with tile.TileContext(nc) as tc:
        with tc.tile_pool(name="main", bufs=1) as pool, \
             tc.tile_pool(name="ypool", bufs=1) as ypool, \
             tc.tile_pool(name="ps_bd", bufs=6, space="PSUM") as ps_bd, \
             tc.tile_pool(name="ps_cv", bufs=1, space="PSUM") as ps_cv, \
             tc.tile_pool(name="ps_w2", bufs=1, space="PSUM") as ps_w2:

            blob_s = pool.tile([P, BW], BF16, tag="blob")
            # conv consts (wconv/ones/bias) first: tiny, unblocks conv
            nc.sync.dma_start(blob_s[:, WCONV_O:BIAS_O + 6],
                              blob_h.ap()[:, WCONV_O:BIAS_O + 6])
            ident = blob_s[:, IDENT_O:IDENT_O + P]

            # ---- x loads: xT halves + xb quarters, split SP/Act issue ----
            xbq = [pool.tile([P, 8 * C], BF16, tag=f"xb{d}", name=f"xb{d}")
                   for d in range(4)]
            x_h = x_in.tensor
            xT = [pool.tile([P, XT_W], BF16, tag=f"xT{g}", name=f"xT{g}")
                  for g in range(2)]
            for g in range(2):
                nc.gpsimd.memset(xT[g][:, 0:1], 0.0)
                nc.gpsimd.memset(xT[g][:, XT_W - 1:XT_W], 0.0)
            XSPL = 2050  # conv quad 3 reads xT cols up to 2050

            def xt_dma(h, g):
                eng = nc.sync if g == 0 else nc.scalar
                lo, hi = (0, XSPL) if h == 0 else (XSPL, L)
                eng.dma_start(xT[g][:, 1 + lo: 1 + hi],
                              xt_in[g * P:(g + 1) * P, lo:hi])

            def xb_dma(d_, eng):
                dst = AP(xbq[d_][:].tensor, 0, [[8 * C, P], [C, 8], [1, C]])
                src = AP(x_h, d_ * 8 * P * C, [[C, P], [P * C, 8], [1, C]])
                eng.dma_start(dst, src)

            def xb_view(m):
                return xbq[m // 8][:, (m % 8) * C:(m % 8 + 1) * C]

            xt_dma(0, 0)
            xt_dma(0, 1)
            # ident + shift matrices (needed by w2/band, not conv)
            nc.scalar.dma_start(blob_s[:, IDENT_O:IDENT_O + P],
                                blob_h.ap()[:, IDENT_O:IDENT_O + P])
            xb_dma(0, nc.sync)
            nc.scalar.dma_start(blob_s[:, SH_O:BW], blob_h.ap()[:, SH_O:BW])
            xt_dma(1, 0)
            xb_dma(1, nc.scalar)
            xt_dma(1, 1)
            xb_dma(2, nc.sync)
            xb_dma(3, nc.scalar)

            # ---- conv -> zT6 [128, NT*6] fp32 (l-part) ----
            zT6 = pool.tile([P, NT * 6], F32, tag="zT6")
            ones_l = blob_s[0:1, ONES_O:ONES_O + P]
            bias_r = blob_s[0:1, BIAS_O:BIAS_O + 6]
            pz_tiles = {}

            def conv_mm(q):
                pz = ps_cv.tile([P, 24], F32, tag="pz")
                pz_tiles[q] = pz
                for t in range(4):
                    m = 4 * q + t
                    dst = pz[:, 6 * t:6 * t + 6]
                    nc.tensor.matmul(dst, ones_l, bias_r,
                                     start=True, stop=False,
                                     skip_group_check=True)
                    n = 0
                    for g in range(2):
                        for dk in range(K):
                            lhsT = xT[g][:, 1 + P * m + dk - 1:
                                         1 + P * m + dk - 1 + P]
                            rhs = blob_s[:, WCONV_O + (g * 3 + dk) * 6:
                                         WCONV_O + (g * 3 + dk) * 6 + 6]
                            n += 1
                            nc.tensor.matmul(dst, lhsT, rhs, start=False,
                                             stop=(n == 6),
                                             skip_group_check=True)

            def conv_drain(q):
                nc.vector.tensor_copy(zT6[:, 24 * q:24 * q + 24],
                                      pz_tiles[q][:])

            # ---- elementwise + vv2 + W2pre, per half ----
            iota = pool.tile([P, NT], F32, tag="iota")
            nc.gpsimd.iota(iota[:], pattern=[[P, NT]], base=0,
                           channel_multiplier=1,
                           allow_small_or_imprecise_dtypes=True)
            spat = pool.tile([P, 9], F32, tag="spat")
            nc.gpsimd.iota(spat[:], pattern=[[1, 9]], base=-4,
                           channel_multiplier=0,
                           allow_small_or_imprecise_dtypes=True)

            pos = pool.tile([P, NT * 3], F32, tag="pos")
            fl = pool.tile([P, NT * 3], F32, tag="fl")
            gt = pool.tile([P, NT * 3], F32, tag="gt")
            alp = pool.tile([P, NT * 3], F32, tag="alp")
            dd = pool.tile([P, NT * 3], F32, tag="dd")
            msk = pool.tile([P, NT * 3], F32, tag="msk")
            wc = pool.tile([P, NT * 3], F32, tag="wc")
            wf = pool.tile([P, NT * 3], F32, tag="wf")
            vv2 = pool.tile([P, ND * NT], BF16, tag="vv2")
            vv2b = pool.tile([P, ND * NT], BF16, tag="vv2b")
            eqa = pool.tile([P, ND * 9], F32, tag="eqa")
            eqb = pool.tile([P, ND * 9], F32, tag="eqb")
            w2pre = pool.tile([P, ND * NT], F32, tag="w2pre")
            # uniform ew chunks; w2 m-chunks shifted so w2(p) (carries touch
            # vv2 t in [m0-1, m0+nm+1)) needs only ew chunks p-1, p
            EW_T = [(0, 8), (8, 8), (16, 8), (24, 8)]
            W2_M = [(0, 7), (7, 8), (15, 8), (23, 9)]

            def elementwise_chunk(p_):
                t0, n_t = EW_T[p_]
                el = [[NT * 3, P], [3, n_t], [1, 3]]
                zoff = AP(zT6[:].tensor, 6 * t0, [[NT * 6, P], [6, n_t], [1, 3]])
                zmsk = AP(zT6[:].tensor, 6 * t0 + 3,
                          [[NT * 6, P], [6, n_t], [1, 3]])
                io_v = AP(iota[:].tensor, t0, [[NT, P], [1, n_t], [0, 3]])
                pos_v = AP(pos[:].tensor, 3 * t0, el)
                fl_v = AP(fl[:].tensor, 3 * t0, el)
                gt_v = AP(gt[:].tensor, 3 * t0, el)
                alp_v = AP(alp[:].tensor, 3 * t0, el)
                dd_v = AP(dd[:].tensor, 3 * t0, el)
                msk_v = AP(msk[:].tensor, 3 * t0, el)
                wc_v = AP(wc[:].tensor, 3 * t0, el)
                wf_v = AP(wf[:].tensor, 3 * t0, el)
                nc.vector.tensor_tensor(pos_v, zoff, io_v, A.add)
                nc.vector.tensor_scalar(pos_v, pos_v, 0.0, float(L - 1),
                                        A.max, A.min)
                nc.vector.tensor_scalar(fl_v, pos_v, 8388608.0, 8388608.0,
                                        A.add, A.subtract)
                nc.vector.tensor_tensor(gt_v, fl_v, pos_v, A.is_gt)
                nc.vector.tensor_tensor(fl_v, fl_v, gt_v, A.subtract)
                nc.vector.tensor_tensor(alp_v, pos_v, fl_v, A.subtract)
                nc.vector.tensor_tensor(dd_v, fl_v, io_v, A.subtract)
                nc.scalar.activation(msk_v, zmsk, ACT.Sigmoid)
                nc.vector.tensor_tensor(wc_v, msk_v, alp_v, A.mult)
                nc.vector.tensor_tensor(wf_v, msk_v, wc_v, A.subtract)

                # vv2: two parallel accumulators (DVE + Pool), then merge
                streams = [(o, so, w) for o in range(3)
                           for so, w in ((1, wf), (0, wc))]
                for ei, (eng, acc, eq) in enumerate(
                        ((nc.vector, vv2, eqa), (nc.vector, vv2b, eqb))):
                    acc_v = AP(acc[:].tensor, t0, [[ND * NT, P], [NT, ND], [1, n_t]])
                    eq_v = AP(eq[:].tensor, 0, [[ND * 9, P], [9, ND], [1, n_t]])
                    first = True
                    for o, so, w in streams[ei::2]:
                        d3 = AP(dd[:].tensor, 3 * t0 + o,
                                [[NT * 3, P], [0, ND], [3, n_t]])
                        w3 = AP(w[:].tensor, 3 * t0 + o,
                                [[NT * 3, P], [0, ND], [3, n_t]])
                        sp3 = AP(spat[:].tensor, so, [[9, P], [1, ND], [0, n_t]])
                        eng.tensor_tensor(eq_v, d3, sp3, A.is_equal)
                        if first:
                            eng.tensor_tensor(acc_v, eq_v, w3, A.mult)
                            first = False
                        else:
                            eng.tensor_tensor(eq_v, eq_v, w3, A.mult)
                            eng.tensor_tensor(acc_v, acc_v, eq_v, A.add)
                va = AP(vv2[:].tensor, t0, [[ND * NT, P], [NT, ND], [1, n_t]])
                vb = AP(vv2b[:].tensor, t0, [[ND * NT, P], [NT, ND], [1, n_t]])
                nc.vector.tensor_tensor(va, va, vb, A.add)

            def w2chunk(p_):
                # W2pre[p, u*NT + m] for m in [m0, m0+nm); carries read vv2
                # cols m0-1 .. m0+nm (within ew chunks p-1, p).
                m0, nm = W2_M[p_]
                pw = ps_w2.tile([P, ND * 9], F32, tag="pw")
                for u in range(ND):
                    si = 6 - u
                    sh = u - 3
                    main_l = blob_s[:, sh_cols[("m", u)]:sh_cols[("m", u)] + P]
                    nc.tensor.matmul(pw[:, u * nm:u * nm + nm], main_l,
                                     vv2[:, si * NT + m0: si * NT + m0 + nm],
                                     start=True, stop=(sh == 0),
                                     skip_group_check=True)
                    if sh == 0:
                        continue
                    car_l = blob_s[:, sh_cols[("c", u)]:sh_cols[("c", u)] + P]
                    if sh > 0:
                        hi = nm if p_ < 3 else nm - 1
                        nc.tensor.matmul(pw[:, u * nm: u * nm + hi], car_l,
                                         vv2[:, si * NT + m0 + 1:
                                             si * NT + m0 + 1 + hi],
                                         start=False, stop=True,
                                         skip_group_check=True)
                    else:
                        lo = 1 if p_ == 0 else 0
                        nc.tensor.matmul(pw[:, u * nm + lo: u * nm + nm], car_l,
                                         vv2[:, si * NT + m0 + lo - 1:
                                             si * NT + m0 + nm - 1],
                                         start=False, stop=True,
                                         skip_group_check=True)
                dst = AP(w2pre[:].tensor, m0, [[ND * NT, P], [NT, ND], [1, nm]])
                src = AP(pw[:].tensor, 0, [[ND * 9, P], [nm, ND], [1, nm]])
                nc.vector.tensor_copy(dst, src)

            # ---- band helpers ----
            y_tiles = {}

            def scalings(q):
                for u in (3, 2, 4, 1, 5, 0, 6):
                    for t in range(4):
                        m = 4 * q + t
                        xv = xb_view(m)
                        y = ypool.tile([P, C], BF16, tag=f"y{u}_{m % 12}",
                                       name=f"y{u}_{m % 12}")
                        w_ap = w2pre[:, u * NT + m: u * NT + m + 1]
                        e = y_eng[(u, m)]
                        if e == "D":
                            nc.vector.tensor_scalar(y[:], xv, w_ap, None, A.mult)
                        elif e == "A":
                            nc.scalar.activation(y[:], xv, ACT.Copy, scale=w_ap)
                        else:
                            nc.gpsimd.tensor_scalar(y[:], xv, w_ap, None, A.mult)
                        y_tiles[(u, m)] = y

            out_cl = [pool.tile([P, L], BF16, tag=f"ocl{g}", name=f"ocl{g}")
                      for g in range(2)]

            prev_ps = [None, None]

            def band(q):
                # per-m psum [128, F]: all 7 mms cover the full F window via
                # onehot rhs OH_u[p, f] = 1[f == p+u] (value lands at
                # l = 128m + p - s, f = l - 128m + 3). Seams resolved at
                # drain exactly like the v2 kernel.
                for t in range(4):
                    m = 4 * q + t
                    for g in range(2):
                        ps = ps_bd.tile([P, F], F32, tag="pb")
                        for i, u in enumerate((3, 2, 4, 1, 5, 0, 6)):
                            y = y_tiles[(u, m)]
                            oh = blob_s[:, OH_O + u * F:OH_O + (u + 1) * F]
                            nc.tensor.matmul(ps[:], y[:, g * P:(g + 1) * P],
                                             oh, start=(i == 0), stop=(i == 6),
                                             skip_group_check=True)
                        dst = out_cl[g][:, m * P:(m + 1) * P]
                        if (m + g) % 2 == 0:
                            nc.scalar.copy(dst, ps[:, 3:131])
                        else:
                            nc.vector.tensor_copy(dst, ps[:, 3:131])
                        seng = nc.vector
                        if m > 0:
                            sl = out_cl[g][:, m * P - 3: m * P]
                            seng.tensor_tensor(sl, sl, ps[:, 0:3], A.add)
                        if prev_ps[g] is not None:
                            sr = out_cl[g][:, m * P: m * P + 3]
                            seng.tensor_tensor(sr, sr,
                                               prev_ps[g][:, 131:134], A.add)
                        prev_ps[g] = ps

            def out_dma(qp):  # quad pair qp in 0..4
                for g in range(2):
                    nc.sync.dma_start(
                        out_d[g * P:(g + 1) * P, 1024 * qp:1024 * (qp + 1)],
                        out_cl[g][:, 1024 * qp:1024 * (qp + 1)])

            # ---- schedule: conv chunks threaded with DMA arrivals; later
            # B-chunks threaded between early scal/band quads ----
            conv_mm(0)
            conv_mm(1)
            conv_drain(0)
            conv_drain(1)
            elementwise_chunk(0)
            conv_mm(2)
            conv_mm(3)
            conv_drain(2)
            conv_drain(3)
            elementwise_chunk(1)
            w2chunk(0)
            scalings(0)
            if dbg:
                for u_ in range(ND):
                    nc.sync.dma_start(y_d[:, u_ * C:(u_ + 1) * C],
                                      y_tiles[(u_, 1)][:])
            w2chunk(1)
            scalings(1)
            band(0)
            conv_mm(4)
            conv_mm(5)
            conv_drain(4)
            conv_drain(5)
            elementwise_chunk(2)
            w2chunk(2)
            scalings(2)
            band(1)
            conv_mm(6)
            conv_mm(7)
            conv_drain(6)
            conv_drain(7)
            scalings(3)
            band(2)
            out_dma(0)
            elementwise_chunk(3)
            w2chunk(3)
            scalings(4)
            band(3)
            scalings(5)
            band(4)
            out_dma(1)
            scalings(6)
            band(5)
            scalings(7)
            band(6)
            out_dma(2)
            band(7)
            out_dma(3)
            if dbg:
                nc.sync.dma_start(z_d[:, :], zT6[:])
                nc.sync.dma_start(w2_d[:, :], w2pre[:])

    nc.compile()
    return nc


def _get_nc(w_off, b_off, w_mask, b_mask):
    key = (w_off.tobytes(), b_off.tobytes(), w_mask.tobytes(), b_mask.tobytes())
    if key not in _cache:
        _cache[key] = _build(w_off, b_off, w_mask, b_mask)
    return _cache[key]


def kernel(x, w_off, b_off, w_mask, b_mask):
    x = np.ascontiguousarray(np.asarray(x, dtype=np.float32))
    xbf = x.astype(ml_dtypes.bfloat16)
    nc = _get_nc(np.asarray(w_off, np.float32), np.asarray(b_off, np.float32),
                 np.asarray(w_mask, np.float32), np.asarray(b_mask, np.float32))
    in_maps = [{"x": xbf[b], "xt": np.ascontiguousarray(xbf[b].T)}
               for b in range(B)]
    res = run_bass_kernel_spmd(nc, in_maps, list(range(B)))
    # out_d is the (C, L) buffer; reference returns its raw (L, C) reshape
    return np.stack([np.asarray(res.results[b]["out"], dtype=np.float32)
                     .reshape(L, C) for b in range(B)])
